# revision 16
# baseline (speedup 1.0000x reference)
"""Trainium2 Bass kernel for nn_MultiHeadDiffAttention (B=2,T=2048,C=1024,H=16).

Sharding: 8 cores = data-parallel over B(2) x tensor-parallel over 4 head-groups
(4 heads each). Each core computes q/k/v projections for its heads, causal
differential attention, per-head GroupNorm (folded into the output projection
weights), and two partial output projections (heads 0-1 and heads 2-3). Host
sums the 8 partials per batch.

Layout strategy per core (v2, S^T formulation):
  - qT/kT per head [64, T]; v in [t, d] tiles [128, 4*129] with a ones column
    per head (gives softmax denominators from the z matmul for free).
  - S^T tiles [k=128, q<=512] computed directly on PE (lhsT = kT k-block,
    rhs = qT q-block) -> exp on ACT -> e^T tiles bf16.
  - z[q, 129] = e^T.T @ [v|1] accumulated over k blocks in PSUM; columns
    0:128 are z, column 128 is the denominator D. Normalization + combine
    (z1/D1 - lam*z2/D2) is a small per-q-tile DVE pass on z instead of two
    passes over the full attention matrix.
  - z tiles transposed on PE to yT [d, t]; GroupNorm stats accumulated via
    ACT copy w/ accumulator (sum) + DVE tensor_tensor_reduce (sum of sq).
  - GroupNorm affine is folded into Wc: wcs = wct * a_d (per-partition DVE
    scale) and a rank-1 bias term computed with tiny PE matmuls, applied
    per-o-partition during the output copy.
  - Output projection is split into two head groups; group A (heads 0-1) is
    interleaved into heads 2-3's attention as PE filler work, group B runs
    at the tail. Host sums both partials.
  - PE work is emitted software-pipelined: z matmuls for score block i are
    emitted after the S matmuls of block i+1, so PE never waits on ACT exp.
"""

import sys

for _p in ("/opt/trn_rl_repo", "/root/.axon_site/_ro/trn_rl_repo"):
    if _p not in sys.path:
        sys.path.insert(0, _p)

import math
import numpy as np
import ml_dtypes

import concourse.bass as bass
import concourse.bacc as bacc
import concourse.tile as tile
import concourse.mybir as mybir
from concourse import bass_utils

F32 = mybir.dt.float32
BF16 = mybir.dt.bfloat16
AF = mybir.ActivationFunctionType
ALU = mybir.AluOpType

B, T, C = 2, 2048, 1024
H = 16
HS = C // H           # 64
D = 2 * HS            # 128 v-channels per head
NH = 4                # heads per core
N_CORES = 8
NT = T // 128         # 16 k/q tiles
LAMBDA_INIT = 0.8 - 0.6 * math.exp(-0.3 * (12 - 1))
EPS = 1e-5
SCALE = 1.0 / math.sqrt(HS)
NEG = -30000.0
VW = 130              # v cols/head: 128 data + ones + pad (even width)

_cache = {}


def _build(trace_sim=False, stage="full"):
    nc = bacc.Bacc("TRN2", target_bir_lowering=False, debug=False,
                   num_devices=N_CORES)

    def din(name, shape, dt=BF16):
        return nc.dram_tensor(name, shape, dt, kind="ExternalInput").ap()

    xT_d = din("xT", [C, T])
    wq1_d = din("wq1T", [C, NH * HS])
    wq2_d = din("wq2T", [C, NH * HS])
    wk1_d = din("wk1T", [C, NH * HS])
    wk2_d = din("wk2T", [C, NH * HS])
    wv_d = din("wvT", [C, NH * D])
    wc_d = din("wcT", [NH * D, C])
    mask01_d = din("mask01", [128, 128])
    ident_d = din("ident", [128, 128])
    gg_d = din("gg", [128, 128], F32)
    gw2_d = din("gw2", [128, 1], F32)
    gb2_d = din("gb2", [128, 1], F32)
    lamn_d = din("lamn", [128, NH], F32)
    icon_d = din("icon", [128, 2], mybir.dt.uint32)
    outA_d = nc.dram_tensor("outA", [C, T], BF16, kind="ExternalOutput").ap()
    outB_d = nc.dram_tensor("outB", [C, T], BF16, kind="ExternalOutput").ap()
    out_ds = [outA_d, outB_d]

    with tile.TileContext(nc, trace_sim=trace_sim) as tc:
        with tc.tile_pool(name="persist", bufs=1) as pp, \
             tc.tile_pool(name="ps_s", bufs=2, space="PSUM") as ps_s, \
             tc.tile_pool(name="ps_z", bufs=4, space="PSUM") as ps_z:

            # ---- persistent small tiles (gpsimd ring) ----
            mask01_t = pp.tile([128, 128], BF16, tag="mask01")
            ident_t = pp.tile([128, 128], BF16, tag="ident")
            gg_t = pp.tile([128, 128], F32, tag="gg")
            gw2_t = pp.tile([128, 1], F32, tag="gw2")
            gb2_t = pp.tile([128, 1], F32, tag="gb2")
            lamn_t = pp.tile([128, NH], F32, tag="lamn")
            icon_t = pp.tile([128, 2], mybir.dt.uint32, tag="icon")
            c15_t = pp.tile([128, 1], F32, tag="c15")
            nc.vector.memset(c15_t[:], 1.5)

            # ---- persistent activation tensors ----
            q1t = [pp.tile([128, T], BF16, tag=f"q1t{i}", name=f"q1t{i}") for i in range(2)]
            q2t = [pp.tile([128, T], BF16, tag=f"q2t{i}", name=f"q2t{i}") for i in range(2)]
            k1t = [pp.tile([128, T], BF16, tag=f"k1t{i}", name=f"k1t{i}") for i in range(2)]
            k2t = [pp.tile([128, T], BF16, tag=f"k2t{i}", name=f"k2t{i}") for i in range(2)]
            # v: [t, NH*VW] with ones col per head (memset to 1.0 first)
            vE = [pp.tile([128, NH * VW], BF16, tag=f"vE{i}", name=f"vE{i}")
                  for i in range(NT)]
            for i in range(NT):
                nc.vector.memset(vE[i][:], 1.0)
            # yT per head [D=128, T] bf16 (RAW, pre-groupnorm-affine)
            ytr = [pp.tile([128, T], BF16, tag=f"ytr{j}", name=f"ytr{j}") for j in range(NH)]
            # wcT: [512, C] as 4 tiles (one per head); wcs = affine-scaled copy
            wct = [pp.tile([128, C], BF16, tag=f"wct{j}", name=f"wct{j}") for j in range(NH)]
            wcs = [pp.tile([128, C], BF16, tag=f"wcs{j}", name=f"wcs{j}") for j in range(NH)]
            bias_sb = [pp.tile([128, 16], F32, tag=f"bias{g}", name=f"bias{g}")
                       for g in range(2)]
            outsb = [pp.tile([128, T], BF16, tag=f"outsb{o}", name=f"outsb{o}")
                     for o in range(8)]

            # ================= projections =================
            with tc.tile_pool(name="loads", bufs=1) as lp:
                xt = [lp.tile([128, T], BF16, tag=f"xt{i}", name=f"xt{i}") for i in range(8)]
                wq = {}
                for nm in ("q1", "q2", "k1", "k2"):
                    wq[nm] = [lp.tile([128, NH * HS], BF16, tag=f"w{nm}{i}",
                                      name=f"w{nm}{i}") for i in range(8)]
                wvt = [lp.tile([128, NH * D], BF16, tag=f"wvt{i}", name=f"wvt{i}")
                       for i in range(8)]
                # DMA issue order: first-needed-first; x split sync/scalar
                for i in range(0, 8, 2):
                    nc.sync.dma_start(xt[i][:], xT_d[i * 128:(i + 1) * 128, :])
                    nc.scalar.dma_start(wq["q1"][i][:],
                                        wq1_d[i * 128:(i + 1) * 128, :])
                    nc.scalar.dma_start(xt[i + 1][:],
                                        xT_d[(i + 1) * 128:(i + 2) * 128, :])
                    nc.scalar.dma_start(wq["q1"][i + 1][:],
                                        wq1_d[(i + 1) * 128:(i + 2) * 128, :])
                for i in range(8):
                    nc.scalar.dma_start(wq["k1"][i][:],
                                        wk1_d[i * 128:(i + 1) * 128, :])
                nc.scalar.dma_start(mask01_t[:], mask01_d)
                nc.scalar.dma_start(ident_t[:], ident_d)
                nc.scalar.dma_start(gg_t[:], gg_d)
                nc.scalar.dma_start(gw2_t[:], gw2_d)
                nc.scalar.dma_start(gb2_t[:], gb2_d)
                nc.scalar.dma_start(lamn_t[:], lamn_d)
                nc.scalar.dma_start(icon_t[:], icon_d)
                for i in range(8):
                    nc.scalar.dma_start(wq["q2"][i][:],
                                        wq2_d[i * 128:(i + 1) * 128, :])
                for i in range(8):
                    nc.scalar.dma_start(wq["k2"][i][:],
                                        wk2_d[i * 128:(i + 1) * 128, :])
                for i in range(8):
                    nc.scalar.dma_start(wvt[i][:], wv_d[i * 128:(i + 1) * 128, :])
                for j in range(NH):
                    nc.sync.dma_start(wct[j][:], wc_d[j * 128:(j + 1) * 128, :])

                cp_state = [0]

                def pcopy(dst, src):
                    # round-robin psum->sbuf copies over ACT / DVE
                    if cp_state[0] % 2 == 0:
                        nc.scalar.copy(dst, src)
                    else:
                        nc.vector.tensor_copy(dst, src)
                    cp_state[0] += 1

                def qk_proj(nm, dst, oc):
                    for half in range(2):
                        ps = ps_s.tile([128, 1024], F32, tag="s", name="psP")
                        for cc in range(8):
                            for sub in range(2):
                                nc.tensor.matmul(
                                    ps[:, sub * 512:(sub + 1) * 512],
                                    wq[nm][cc][:, oc * 128:(oc + 1) * 128],
                                    xt[cc][:, half * 1024 + sub * 512:
                                           half * 1024 + (sub + 1) * 512],
                                    start=(cc == 0), stop=(cc == 7),
                                    skip_group_check=True)
                        pcopy(dst[oc][:, half * 1024:(half + 1) * 1024], ps[:])

                # oc0: q1/k1 cc-outer (overlaps x DMA arrival), then q2/k2
                for nm, dst in (("q1", q1t), ("k1", k1t)):
                    ph = [ps_s.tile([128, 1024], F32, tag="s", name="psP")
                          for _ in range(2)]
                    for cc in range(8):
                        for half in range(2):
                            for sub in range(2):
                                nc.tensor.matmul(
                                    ph[half][:, sub * 512:(sub + 1) * 512],
                                    wq[nm][cc][:, 0:128],
                                    xt[cc][:, half * 1024 + sub * 512:
                                           half * 1024 + (sub + 1) * 512],
                                    start=(cc == 0), stop=(cc == 7),
                                    skip_group_check=True)
                    for half in range(2):
                        pcopy(dst[0][:, half * 1024:(half + 1) * 1024],
                              ph[half][:])
                for nm, dst in (("q2", q2t), ("k2", k2t)):
                    qk_proj(nm, dst, 0)
                for tch in range(NT):
                    ps = ps_s.tile([128, 512], F32, tag="s", name="psV")
                    for cc in range(8):
                        nc.tensor.matmul(
                            ps[:],
                            xt[cc][:, tch * 128:(tch + 1) * 128],
                            wvt[cc][:],
                            start=(cc == 0), stop=(cc == 7))
                    for jj in range(NH):
                        pcopy(vE[tch][:, jj * VW:jj * VW + 128],
                              ps[:, jj * 128:(jj + 1) * 128])
                for nm, dst in (("q1", q1t), ("k1", k1t), ("q2", q2t),
                                ("k2", k2t)):
                    qk_proj(nm, dst, 1)

            # ================= attention =================
            wp_cm = tc.tile_pool(name="aw", bufs=2)
            wp = wp_cm.__enter__()

            P = {"z": None, "fin": [], "fill": []}

            def flush_pending():
                if P["z"] is not None:
                    zf = P["z"]
                    P["z"] = None
                    zf()
                fins = P["fin"]
                P["fin"] = []
                for f in fins:
                    f()
                if P["fill"]:
                    P["fill"].pop(0)()

            def emit_combine(j, zp_q, zcb, qq):
                rr = wp.tile([128, 2], F32, tag="rr", name="rr", bufs=3)
                nc.vector.reciprocal(rr[:, 0:1], zp_q[:, 128:129])
                nc.vector.reciprocal(rr[:, 1:2], zp_q[:, 258:259])
                tmz = wp.tile([128, 128], F32, tag="tmz", name="tmz", bufs=3)
                nc.vector.tensor_scalar(tmz[:], zp_q[:, 130:258], rr[:, 1:2],
                                        lamn_t[:, j:j + 1],
                                        op0=ALU.mult, op1=ALU.mult)
                nc.vector.scalar_tensor_tensor(
                    zcb[:, qq * 128:(qq + 1) * 128], zp_q[:, 0:128],
                    rr[:, 0:1], tmz[:], op0=ALU.mult, op1=ALU.add)

            def make_finish(j, qb, zcb, s1p, s2p, head_post):
                def go():
                    pt = ps_s.tile([128, 512], BF16, tag="s", name="pt")
                    for qq in range(4):
                        nc.tensor.transpose(pt[:, qq * 128:(qq + 1) * 128],
                                            zcb[:, qq * 128:(qq + 1) * 128],
                                            ident_t[:])
                    nc.scalar.activation(
                        ytr[j][:, qb * 512:(qb + 1) * 512], pt[:], AF.Copy,
                        accum_out=s1p[:, qb:qb + 1])
                    sq = wp.tile([128, 512], BF16, tag="sq", name="sq", bufs=2)
                    ysl = ytr[j][:, qb * 512:(qb + 1) * 512]
                    nc.vector.tensor_tensor(sq[:], ysl, ysl, ALU.mult)
                    nc.vector.tensor_reduce(s2p[:, qb:qb + 1], sq[:],
                                            axis=mybir.AxisListType.X,
                                            op=ALU.add)
                    if head_post is not None and LV >= 5:
                        head_post()
                return go

            def make_gn(j, s1p, s2p):
                def go():
                    s12 = wp.tile([128, 2], F32, tag="s12", name="s12")
                    nc.vector.tensor_reduce(s12[:, 0:1], s1p[:, 0:4],
                                            axis=mybir.AxisListType.X, op=ALU.add)
                    nc.vector.tensor_reduce(s12[:, 1:2], s2p[:, 0:4],
                                            axis=mybir.AxisListType.X, op=ALU.add)
                    pg = ps_z.tile([128, 2], F32, tag="z", name="pg")
                    nc.tensor.matmul(pg[:], gg_t[:], s12[:], start=True,
                                     stop=True, skip_group_check=True)
                    mneg = wp.tile([128, 1], F32, tag="mneg", name="mneg")
                    nc.scalar.mul(mneg[:], pg[:, 0:1], -1.0 / (T * 4))
                    msq = wp.tile([128, 1], F32, tag="msq", name="msq")
                    nc.scalar.mul(msq[:], pg[:, 1:2], 1.0 / (T * 4))
                    nvar = wp.tile([128, 1], F32, tag="nvar", name="nvar")
                    nc.vector.scalar_tensor_tensor(
                        nvar[:], mneg[:], mneg[:, 0:1], msq[:],
                        op0=ALU.mult, op1=ALU.subtract)
                    vpe = wp.tile([128, 1], F32, tag="vpe", name="vpe")
                    nc.vector.tensor_scalar(vpe[:], nvar[:], -1.0, EPS,
                                            op0=ALU.mult, op1=ALU.add)
                    # rsqrt(var+eps) on DVE: quake seed + 3 Newton iters
                    yi = wp.tile([128, 1], F32, tag="yi", name="yi")
                    nc.vector.tensor_tensor(yi.bitcast(mybir.dt.uint32)[:],
                                            vpe.bitcast(mybir.dt.uint32)[:],
                                            icon_t[:, 0:1],
                                            ALU.logical_shift_right)
                    nc.vector.tensor_tensor(yi.bitcast(mybir.dt.uint32)[:],
                                            icon_t[:, 1:2],
                                            yi.bitcast(mybir.dt.uint32)[:],
                                            ALU.subtract)
                    vneg = wp.tile([128, 1], F32, tag="vneg", name="vneg")
                    nc.vector.tensor_scalar_mul(vneg[:], vpe[:], -0.5)
                    ytmp = wp.tile([128, 1], F32, tag="ytmp", name="ytmp")
                    for _ in range(3):
                        nc.vector.tensor_tensor(ytmp[:], yi[:], yi[:], ALU.mult)
                        nc.vector.scalar_tensor_tensor(
                            ytmp[:], ytmp[:], vneg[:, 0:1], c15_t[:],
                            op0=ALU.mult, op1=ALU.add)
                        nc.vector.tensor_tensor(yi[:], yi[:], ytmp[:], ALU.mult)
                    aff_a = wp.tile([128, 1], F32, tag="aff_a", name="aff_a")
                    nc.vector.tensor_tensor(aff_a[:], yi[:], gw2_t[:], ALU.mult)
                    affb = wp.tile([128, 2], BF16, tag="affb", name="affb")
                    nc.vector.scalar_tensor_tensor(
                        affb[:, 0:1], mneg[:], aff_a[:, 0:1], gb2_t[:],
                        op0=ALU.mult, op1=ALU.add)  # gb2 - mean*aff_a
                    nc.vector.memset(affb[:, 1:2], 0.0)
                    # fold affine scale into Wc slice for this head
                    nc.vector.tensor_scalar(wcs[j][:], wct[j][:],
                                            aff_a[:, 0:1], None, op0=ALU.mult)
                    gn_done[j] = affb
                    if j % 2 == 1:
                        emit_group(j // 2)
                return go

            def emit_group(g):
                if stage in ("h0", "heads"):
                    return
                js = (0, 1) if g == 0 else (2, 3)
                # rank-1 bias: bias[o] = sum_d wct[d,o] * affb[d], both heads
                bps = ps_z.tile([128, 16], F32, tag="z", name="bps")
                for ocb in range(8):
                    for i, j in enumerate(js):
                        nc.tensor.matmul(bps[:, ocb * 2:(ocb + 1) * 2],
                                         wct[j][:, ocb * 128:(ocb + 1) * 128],
                                         gn_done[j][:, 0:2],
                                         start=(i == 0), stop=(i == 1),
                                         skip_group_check=True)
                nc.vector.tensor_copy(bias_sb[g][:], bps[:])
                units = []
                for ocb in range(8):
                    for tb in range(T // 512):
                        units.append(make_po(g, js, ocb, tb))
                if g == 0:
                    P["fill"].extend(units)
                else:
                    for u in units:
                        u()

            def make_po(g, js, ocb, tb):
                def go():
                    po = ps_s.tile([128, 512], F32, tag="s", name="po")
                    for i, j in enumerate(js):
                        nc.tensor.matmul(
                            po[:],
                            wcs[j][:, ocb * 128:(ocb + 1) * 128],
                            ytr[j][:, tb * 512:(tb + 1) * 512],
                            start=(i == 0), stop=(i == 1),
                            skip_group_check=True)
                    ob = outsb[ocb][:, tb * 512:(tb + 1) * 512]
                    if g == 0 or (ocb + tb) % 2 == 0:
                        nc.vector.tensor_scalar(
                            ob, po[:],
                            bias_sb[g][:, 2 * ocb:2 * ocb + 1], None,
                            op0=ALU.add)
                    else:
                        nc.scalar.activation(
                            ob, po[:], AF.Identity,
                            bias=bias_sb[g][:, 2 * ocb:2 * ocb + 1])
                    if tb == 3:
                        ring = nc.sync if (g == 0 or ocb % 2 == 0)                             else nc.scalar
                        ring.dma_start(out_ds[g][ocb * 128:(ocb + 1) * 128, :],
                                       outsb[ocb][:])
                return go

            gn_done = {}

            LV = {"proj": 0, "sblk": 1, "zmm": 2, "comb": 3, "fin": 4,
                  "h0": 5, "heads": 6, "full": 7}[stage]
            nheads = 0 if LV == 0 else (1 if LV <= 5 else NH)
            for j in range(nheads):
                oc, po_ = divmod(j * HS, 128)
                s1p = wp.tile([128, 4], F32, tag="s1p", name="s1p", bufs=2)
                s2p = wp.tile([128, 4], F32, tag="s2p", name="s2p", bufs=2)
                head_post = make_gn(j, s1p, s2p)
                for qb in range(4):
                    zcb = wp.tile([128, 512], BF16, tag="zcb", name="zcb",
                                  bufs=2)
                    zp = [ps_z.tile([128, 260], F32, tag="z", name=f"zp{qq}")
                          for qq in range(4)]
                    for kt in range(4 * qb + 4):
                        diag = kt >= 4 * qb
                        qoff = (kt - 4 * qb) * 128 if diag else 0
                        ps = ps_s.tile([128, 2, 512], F32, tag="s", name="psS")
                        nc.tensor.matmul(
                            ps[:, 0, qoff:512],
                            k1t[oc][po_:po_ + HS, kt * 128:(kt + 1) * 128],
                            q1t[oc][po_:po_ + HS,
                                    qb * 512 + qoff:(qb + 1) * 512],
                            start=True, stop=True, skip_group_check=True)
                        nc.tensor.matmul(
                            ps[:, 1, qoff:512],
                            k2t[oc][po_:po_ + HS, kt * 128:(kt + 1) * 128],
                            q2t[oc][po_:po_ + HS,
                                    qb * 512 + qoff:(qb + 1) * 512],
                            start=True, stop=True, skip_group_check=True)
                        flush_pending()
                        e = wp.tile([128, 2, 512], BF16, tag="e", name="e",
                                    bufs=3)
                        nc.scalar.activation(e[:, :, qoff:512],
                                             ps[:, :, qoff:512],
                                             AF.Exp, scale=SCALE)
                        if diag:
                            # causal mask: zero e where q < k (multiplicative,
                            # safe: no inf in exp range here)
                            for m in range(2):
                                nc.vector.tensor_tensor(
                                    e[:, m, qoff:qoff + 128],
                                    e[:, m, qoff:qoff + 128],
                                    mask01_t[:], ALU.mult)

                        def make_z(j=j, qb=qb, kt=kt, e=e, zp=zp, zcb=zcb,
                                   s1p=s1p, s2p=s2p, head_post=head_post):
                            def go():
                                if LV < 2:
                                    return
                                for qt in range(max(kt, 4 * qb), 4 * qb + 4):
                                    qq = qt - 4 * qb
                                    for m in range(2):
                                        # start only the bank's first matmul:
                                        # start_tensor_calc marks the WHOLE
                                        # 2KB zero region pending-zero, so
                                        # z2's kt==0 write must rely on z1's
                                        # start (it zeroes on first touch).
                                        nc.tensor.matmul(
                                            zp[qq][:, m * 130:m * 130 + 130],
                                            e[:, m, qq * 128:qq * 128 + 128],
                                            vE[kt][:, j * VW:(j + 1) * VW],
                                            start=(kt == 0 and m == 0),
                                            stop=(kt == qt),
                                            skip_group_check=True)
                                if kt >= 4 * qb and LV >= 3:
                                    qq = kt - 4 * qb
                                    emit_combine(j, zp[qq], zcb, qq)
                                    if qq == 3 and LV >= 4:
                                        P["fin"].append(make_finish(
                                            j, qb, zcb, s1p, s2p,
                                            head_post if qb == 3 else None))
                            return go

                        P["z"] = make_z()

            # drain: last z block, last finish (incl. gn(3) -> emits group B)
            flush_pending()
            flush_pending()
            while P["fill"]:
                P["fill"].pop(0)()
            if stage != "full":
                # dump something defined to the outputs so the run completes
                for g, src_t in ((0, q1t[0]), (1, k1t[0])):
                    if LV >= 4:
                        src_t = ytr[0]
                    dmp = wp.tile([128, T], F32, tag="dmp", name="dmp", bufs=2)
                    nc.vector.tensor_copy(dmp[:], src_t[:])
                    nc.sync.dma_start(out_ds[g][0:128, :], dmp[:])

            wp_cm.__exit__(None, None, None)

    nc.compile()
    return nc


def _prep_inputs(inputs):
    bf = ml_dtypes.bfloat16
    x = np.asarray(inputs["x"], np.float32)
    Wq1 = np.asarray(inputs["Wq1"], np.float32)
    Wq2 = np.asarray(inputs["Wq2"], np.float32)
    Wk1 = np.asarray(inputs["Wk1"], np.float32)
    Wk2 = np.asarray(inputs["Wk2"], np.float32)
    Wv = np.asarray(inputs["Wv"], np.float32)
    Wc = np.asarray(inputs["Wc"], np.float32)
    gn_w = np.asarray(inputs["gn_w"], np.float32)
    gn_b = np.asarray(inputs["gn_b"], np.float32)
    gamma = np.asarray(inputs["gamma"], np.float32)

    def sig(v):
        return 1.0 / (1.0 + np.exp(-v))

    lam = (sig(np.asarray(inputs["lq1"], np.float32).reshape(H)
               * np.asarray(inputs["lk1"], np.float32).reshape(H))
           - sig(np.asarray(inputs["lq2"], np.float32).reshape(H)
                 * np.asarray(inputs["lk2"], np.float32).reshape(H))
           + LAMBDA_INIT)

    # S^T layout: mask rows k, cols q; keep where q >= k
    mask01 = (np.arange(128)[None, :] >= np.arange(128)[:, None]).astype(
        np.float32).astype(bf)
    ident = np.eye(128, dtype=np.float32).astype(bf)
    gg = (np.arange(128)[:, None] // 4 == np.arange(128)[None, :] // 4
          ).astype(np.float32)
    c1 = 1.0 - LAMBDA_INIT
    gw2 = (gn_w * gamma * c1).astype(np.float32).reshape(128, 1)
    gb2 = (gn_b * gamma * c1).astype(np.float32).reshape(128, 1)

    icon = np.zeros((128, 2), np.uint32)
    icon[:, 0] = 1
    icon[:, 1] = 0x5f375a00
    xTb = [np.ascontiguousarray(x[b].T).astype(bf) for b in range(B)]
    in_maps = []
    for core in range(N_CORES):
        b, hg = divmod(core, N_CORES // B)
        qs = hg * NH * HS          # 256-wide q/k slice
        vs = hg * NH * D           # 512-wide v / y2 slice
        lamn = np.repeat(-lam[hg * NH:(hg + 1) * NH].reshape(1, NH),
                         128, axis=0).astype(np.float32)
        in_maps.append({
            "xT": xTb[b],
            "wq1T": np.ascontiguousarray(Wq1[qs:qs + NH * HS, :].T).astype(bf),
            "wq2T": np.ascontiguousarray(Wq2[qs:qs + NH * HS, :].T).astype(bf),
            "wk1T": np.ascontiguousarray(Wk1[qs:qs + NH * HS, :].T).astype(bf),
            "wk2T": np.ascontiguousarray(Wk2[qs:qs + NH * HS, :].T).astype(bf),
            "wvT": np.ascontiguousarray(Wv[vs:vs + NH * D, :].T).astype(bf),
            "wcT": np.ascontiguousarray(Wc[:, vs:vs + NH * D].T).astype(bf),
            "mask01": mask01,
            "ident": ident,
            "gg": gg,
            "gw2": gw2,
            "gb2": gb2,
            "lamn": lamn,
            "icon": icon,
        })
    return in_maps


def kernel(**inputs):
    if "nc" not in _cache:
        _cache["nc"] = _build()
    nc = _cache["nc"]
    in_maps = _prep_inputs(inputs)
    res = bass_utils.run_bass_kernel_spmd(
        nc, in_maps, core_ids=list(range(N_CORES)),
        **_cache.get("run_kwargs", {}))
    _cache["last_result"] = res
    out = np.zeros((B, T, C), np.float32)
    for core in range(N_CORES):
        b = core // (N_CORES // B)
        out[b] += res.results[core]["outA"].astype(np.float32).T
        out[b] += res.results[core]["outB"].astype(np.float32).T
    return out


# revision 17
# speedup vs baseline: 1.0199x; 1.0199x over previous
"""Trainium2 Bass kernel for nn_MultiHeadDiffAttention (B=2,T=2048,C=1024,H=16).

Sharding: 8 cores = data-parallel over B(2) x tensor-parallel over 4 head-groups
(4 heads each). Each core computes q/k/v projections for its heads, causal
differential attention, per-head GroupNorm (folded into the output projection
weights), and two partial output projections (heads 0-1 and heads 2-3). Host
sums the 8 partials per batch.

Layout strategy per core (v2, S^T formulation):
  - qT/kT per head [64, T]; v in [t, d] tiles [128, 4*129] with a ones column
    per head (gives softmax denominators from the z matmul for free).
  - S^T tiles [k=128, q<=512] computed directly on PE (lhsT = kT k-block,
    rhs = qT q-block) -> exp on ACT -> e^T tiles bf16.
  - z[q, 129] = e^T.T @ [v|1] accumulated over k blocks in PSUM; columns
    0:128 are z, column 128 is the denominator D. Normalization + combine
    (z1/D1 - lam*z2/D2) is a small per-q-tile DVE pass on z instead of two
    passes over the full attention matrix.
  - z tiles transposed on PE to yT [d, t]; GroupNorm stats accumulated via
    ACT copy w/ accumulator (sum) + DVE tensor_tensor_reduce (sum of sq).
  - GroupNorm affine is folded into Wc: wcs = wct * a_d (per-partition DVE
    scale) and a rank-1 bias term computed with tiny PE matmuls, applied
    per-o-partition during the output copy.
  - Output projection is split into two head groups; group A (heads 0-1) is
    interleaved into heads 2-3's attention as PE filler work, group B runs
    at the tail. Host sums both partials.
  - PE work is emitted software-pipelined: z matmuls for score block i are
    emitted after the S matmuls of block i+1, so PE never waits on ACT exp.
"""

import sys

for _p in ("/opt/trn_rl_repo", "/root/.axon_site/_ro/trn_rl_repo"):
    if _p not in sys.path:
        sys.path.insert(0, _p)

import math
import numpy as np
import ml_dtypes

import concourse.bass as bass
import concourse.bacc as bacc
import concourse.tile as tile
import concourse.mybir as mybir
from concourse import bass_utils

F32 = mybir.dt.float32
BF16 = mybir.dt.bfloat16
AF = mybir.ActivationFunctionType
ALU = mybir.AluOpType

B, T, C = 2, 2048, 1024
H = 16
HS = C // H           # 64
D = 2 * HS            # 128 v-channels per head
NH = 4                # heads per core
N_CORES = 8
NT = T // 128         # 16 k/q tiles
LAMBDA_INIT = 0.8 - 0.6 * math.exp(-0.3 * (12 - 1))
EPS = 1e-5
SCALE = 1.0 / math.sqrt(HS)
NEG = -30000.0
VW = 130              # v cols/head: 128 data + ones + pad (even width)

_cache = {}


def _build(trace_sim=False, stage="full"):
    nc = bacc.Bacc("TRN2", target_bir_lowering=False, debug=False,
                   num_devices=N_CORES)

    def din(name, shape, dt=BF16):
        return nc.dram_tensor(name, shape, dt, kind="ExternalInput").ap()

    xT_d = din("xT", [C, T])
    wq1_d = din("wq1T", [C, NH * HS])
    wq2_d = din("wq2T", [C, NH * HS])
    wk1_d = din("wk1T", [C, NH * HS])
    wk2_d = din("wk2T", [C, NH * HS])
    wv_d = din("wvT", [C, NH * D])
    wc_d = din("wcT", [NH * D, C])
    mask01_d = din("mask01", [128, 128])
    ident_d = din("ident", [128, 128])
    gg_d = din("gg", [128, 128], F32)
    gw2_d = din("gw2", [128, 1], F32)
    gb2_d = din("gb2", [128, 1], F32)
    lamn_d = din("lamn", [128, NH], F32)
    icon_d = din("icon", [128, 2], mybir.dt.uint32)
    outA_d = nc.dram_tensor("outA", [C, T], BF16, kind="ExternalOutput").ap()
    outB_d = nc.dram_tensor("outB", [C, T], BF16, kind="ExternalOutput").ap()
    out_ds = [outA_d, outB_d]

    with tile.TileContext(nc, trace_sim=trace_sim) as tc:
        with tc.tile_pool(name="persist", bufs=1) as pp, \
             tc.tile_pool(name="ps_s", bufs=2, space="PSUM") as ps_s, \
             tc.tile_pool(name="ps_z", bufs=4, space="PSUM") as ps_z:

            # ---- persistent small tiles (gpsimd ring) ----
            mask01_t = pp.tile([128, 128], BF16, tag="mask01")
            ident_t = pp.tile([128, 128], BF16, tag="ident")
            gg_t = pp.tile([128, 128], F32, tag="gg")
            gw2_t = pp.tile([128, 1], F32, tag="gw2")
            gb2_t = pp.tile([128, 1], F32, tag="gb2")
            lamn_t = pp.tile([128, NH], F32, tag="lamn")
            icon_t = pp.tile([128, 2], mybir.dt.uint32, tag="icon")
            c15_t = pp.tile([128, 1], F32, tag="c15")
            nc.vector.memset(c15_t[:], 1.5)

            # ---- persistent activation tensors ----
            q1t = [pp.tile([128, T], BF16, tag=f"q1t{i}", name=f"q1t{i}") for i in range(2)]
            q2t = [pp.tile([128, T], BF16, tag=f"q2t{i}", name=f"q2t{i}") for i in range(2)]
            k1t = [pp.tile([128, T], BF16, tag=f"k1t{i}", name=f"k1t{i}") for i in range(2)]
            k2t = [pp.tile([128, T], BF16, tag=f"k2t{i}", name=f"k2t{i}") for i in range(2)]
            # v: [t, NH*VW] with ones col per head (memset to 1.0 first)
            vE = [pp.tile([128, NH * VW], BF16, tag=f"vE{i}", name=f"vE{i}")
                  for i in range(NT)]
            for i in range(NT):
                nc.vector.memset(vE[i][:], 1.0)
            # yT per head [D=128, T] bf16 (RAW, pre-groupnorm-affine)
            ytr = [pp.tile([128, T], BF16, tag=f"ytr{j}", name=f"ytr{j}") for j in range(NH)]
            # wcT: [512, C] as 4 tiles (one per head); wcs = affine-scaled copy
            wct_all = pp.tile([128, NH, C], BF16, tag="wct", name="wct_all")
            wct = [wct_all[:, j, :] for j in range(NH)]
            wcs = [pp.tile([128, C], BF16, tag=f"wcs{j}", name=f"wcs{j}") for j in range(NH)]
            bias_sb = [pp.tile([128, 16], F32, tag=f"bias{g}", name=f"bias{g}")
                       for g in range(2)]
            outsb = [pp.tile([128, T], BF16, tag=f"outsb{o}", name=f"outsb{o}")
                     for o in range(8)]

            # ================= projections =================
            with tc.tile_pool(name="loads", bufs=1) as lp:
                xt_all = lp.tile([128, 8, T], BF16, tag="xt", name="xt_all")
                xt = [xt_all[:, i, :] for i in range(8)]
                wq = {}
                wq_all = {}
                for nm in ("q1", "q2", "k1", "k2"):
                    wq_all[nm] = lp.tile([128, 8, NH * HS], BF16, tag=f"w{nm}",
                                         name=f"w{nm}_all")
                    wq[nm] = [wq_all[nm][:, i, :] for i in range(8)]
                wvt_all = lp.tile([128, 8, NH * D], BF16, tag="wvt",
                                  name="wvt_all")
                wvt = [wvt_all[:, i, :] for i in range(8)]
                # Few, big DMAs (each issue costs ~600ns of queue time):
                # x in 4 two-chunk pieces on sync, weights whole on scalar.
                xr = xT_d.rearrange("(c p) t -> p c t", p=128)
                for i in range(4):
                    nc.sync.dma_start(xt_all[:, 2 * i:2 * i + 2, :],
                                      xr[:, 2 * i:2 * i + 2, :])
                for nm, d_ap in (("q1", wq1_d), ("k1", wk1_d),
                                 ("q2", wq2_d), ("k2", wk2_d)):
                    nc.scalar.dma_start(
                        wq_all[nm][:],
                        d_ap.rearrange("(c p) o -> p c o", p=128))
                nc.scalar.dma_start(wvt_all[:],
                                    wv_d.rearrange("(c p) o -> p c o", p=128))
                nc.sync.dma_start(
                    wct_all[:], wc_d.rearrange("(c p) o -> p c o", p=128))
                nc.sync.dma_start(mask01_t[:], mask01_d)
                nc.sync.dma_start(ident_t[:], ident_d)
                nc.sync.dma_start(gg_t[:], gg_d)
                nc.sync.dma_start(gw2_t[:], gw2_d)
                nc.sync.dma_start(gb2_t[:], gb2_d)
                nc.sync.dma_start(lamn_t[:], lamn_d)
                nc.sync.dma_start(icon_t[:], icon_d)

                cp_state = [0]

                def pcopy(dst, src):
                    # round-robin psum->sbuf copies over ACT / DVE
                    if cp_state[0] % 2 == 0:
                        nc.scalar.copy(dst, src)
                    else:
                        nc.vector.tensor_copy(dst, src)
                    cp_state[0] += 1

                def qk_proj(nm, dst, oc):
                    for half in range(2):
                        ps = ps_s.tile([128, 1024], F32, tag="s", name="psP")
                        for cc in range(8):
                            for sub in range(2):
                                nc.tensor.matmul(
                                    ps[:, sub * 512:(sub + 1) * 512],
                                    wq[nm][cc][:, oc * 128:(oc + 1) * 128],
                                    xt[cc][:, half * 1024 + sub * 512:
                                           half * 1024 + (sub + 1) * 512],
                                    start=(cc == 0), stop=(cc == 7),
                                    skip_group_check=True)
                        pcopy(dst[oc][:, half * 1024:(half + 1) * 1024], ps[:])

                # oc0: q1/k1 cc-outer (overlaps x DMA arrival), then q2/k2
                for nm, dst in (("q1", q1t), ("k1", k1t)):
                    ph = [ps_s.tile([128, 1024], F32, tag="s", name="psP")
                          for _ in range(2)]
                    for cc in range(8):
                        for half in range(2):
                            for sub in range(2):
                                nc.tensor.matmul(
                                    ph[half][:, sub * 512:(sub + 1) * 512],
                                    wq[nm][cc][:, 0:128],
                                    xt[cc][:, half * 1024 + sub * 512:
                                           half * 1024 + (sub + 1) * 512],
                                    start=(cc == 0), stop=(cc == 7),
                                    skip_group_check=True)
                    for half in range(2):
                        pcopy(dst[0][:, half * 1024:(half + 1) * 1024],
                              ph[half][:])
                for nm, dst in (("q2", q2t), ("k2", k2t)):
                    qk_proj(nm, dst, 0)
                for tch in range(NT):
                    ps = ps_s.tile([128, 512], F32, tag="s", name="psV")
                    for cc in range(8):
                        nc.tensor.matmul(
                            ps[:],
                            xt[cc][:, tch * 128:(tch + 1) * 128],
                            wvt[cc][:],
                            start=(cc == 0), stop=(cc == 7))
                    for jj in range(NH):
                        pcopy(vE[tch][:, jj * VW:jj * VW + 128],
                              ps[:, jj * 128:(jj + 1) * 128])
                for nm, dst in (("q1", q1t), ("k1", k1t), ("q2", q2t),
                                ("k2", k2t)):
                    qk_proj(nm, dst, 1)

            # ================= attention =================
            wp_cm = tc.tile_pool(name="aw", bufs=2)
            wp = wp_cm.__enter__()

            P = {"z": None, "fin": [], "fill": []}

            def flush_pending():
                if P["z"] is not None:
                    zf = P["z"]
                    P["z"] = None
                    zf()
                fins = P["fin"]
                P["fin"] = []
                for f in fins:
                    f()
                if P["fill"]:
                    P["fill"].pop(0)()

            def emit_combine(j, zp_q, zcb, qq):
                rr = wp.tile([128, 2], F32, tag="rr", name="rr", bufs=3)
                nc.vector.reciprocal(rr[:, 0:1], zp_q[:, 128:129])
                nc.vector.reciprocal(rr[:, 1:2], zp_q[:, 258:259])
                tmz = wp.tile([128, 128], F32, tag="tmz", name="tmz", bufs=3)
                nc.vector.tensor_scalar(tmz[:], zp_q[:, 130:258], rr[:, 1:2],
                                        lamn_t[:, j:j + 1],
                                        op0=ALU.mult, op1=ALU.mult)
                nc.vector.scalar_tensor_tensor(
                    zcb[:, qq * 128:(qq + 1) * 128], zp_q[:, 0:128],
                    rr[:, 0:1], tmz[:], op0=ALU.mult, op1=ALU.add)

            def make_finish(j, qb, zcb, s1p, s2p, head_post):
                def go():
                    pt = ps_s.tile([128, 512], BF16, tag="s", name="pt")
                    for qq in range(4):
                        nc.tensor.transpose(pt[:, qq * 128:(qq + 1) * 128],
                                            zcb[:, qq * 128:(qq + 1) * 128],
                                            ident_t[:])
                    nc.scalar.activation(
                        ytr[j][:, qb * 512:(qb + 1) * 512], pt[:], AF.Copy,
                        accum_out=s1p[:, qb:qb + 1])
                    sq = wp.tile([128, 512], BF16, tag="sq", name="sq", bufs=2)
                    ysl = ytr[j][:, qb * 512:(qb + 1) * 512]
                    nc.vector.tensor_tensor(sq[:], ysl, ysl, ALU.mult)
                    nc.vector.tensor_reduce(s2p[:, qb:qb + 1], sq[:],
                                            axis=mybir.AxisListType.X,
                                            op=ALU.add)
                    if head_post is not None and LV >= 5:
                        head_post()
                return go

            def make_gn(j, s1p, s2p):
                def go():
                    s12 = wp.tile([128, 2], F32, tag="s12", name="s12")
                    nc.vector.tensor_reduce(s12[:, 0:1], s1p[:, 0:4],
                                            axis=mybir.AxisListType.X, op=ALU.add)
                    nc.vector.tensor_reduce(s12[:, 1:2], s2p[:, 0:4],
                                            axis=mybir.AxisListType.X, op=ALU.add)
                    pg = ps_z.tile([128, 2], F32, tag="z", name="pg")
                    nc.tensor.matmul(pg[:], gg_t[:], s12[:], start=True,
                                     stop=True, skip_group_check=True)
                    mneg = wp.tile([128, 1], F32, tag="mneg", name="mneg")
                    nc.scalar.mul(mneg[:], pg[:, 0:1], -1.0 / (T * 4))
                    msq = wp.tile([128, 1], F32, tag="msq", name="msq")
                    nc.scalar.mul(msq[:], pg[:, 1:2], 1.0 / (T * 4))
                    nvar = wp.tile([128, 1], F32, tag="nvar", name="nvar")
                    nc.vector.scalar_tensor_tensor(
                        nvar[:], mneg[:], mneg[:, 0:1], msq[:],
                        op0=ALU.mult, op1=ALU.subtract)
                    vpe = wp.tile([128, 1], F32, tag="vpe", name="vpe")
                    nc.vector.tensor_scalar(vpe[:], nvar[:], -1.0, EPS,
                                            op0=ALU.mult, op1=ALU.add)
                    # rsqrt(var+eps) on DVE: quake seed + 3 Newton iters
                    yi = wp.tile([128, 1], F32, tag="yi", name="yi")
                    nc.vector.tensor_tensor(yi.bitcast(mybir.dt.uint32)[:],
                                            vpe.bitcast(mybir.dt.uint32)[:],
                                            icon_t[:, 0:1],
                                            ALU.logical_shift_right)
                    nc.vector.tensor_tensor(yi.bitcast(mybir.dt.uint32)[:],
                                            icon_t[:, 1:2],
                                            yi.bitcast(mybir.dt.uint32)[:],
                                            ALU.subtract)
                    vneg = wp.tile([128, 1], F32, tag="vneg", name="vneg")
                    nc.vector.tensor_scalar_mul(vneg[:], vpe[:], -0.5)
                    ytmp = wp.tile([128, 1], F32, tag="ytmp", name="ytmp")
                    for _ in range(3):
                        nc.vector.tensor_tensor(ytmp[:], yi[:], yi[:], ALU.mult)
                        nc.vector.scalar_tensor_tensor(
                            ytmp[:], ytmp[:], vneg[:, 0:1], c15_t[:],
                            op0=ALU.mult, op1=ALU.add)
                        nc.vector.tensor_tensor(yi[:], yi[:], ytmp[:], ALU.mult)
                    aff_a = wp.tile([128, 1], F32, tag="aff_a", name="aff_a")
                    nc.vector.tensor_tensor(aff_a[:], yi[:], gw2_t[:], ALU.mult)
                    affb = wp.tile([128, 2], BF16, tag="affb", name="affb")
                    nc.vector.scalar_tensor_tensor(
                        affb[:, 0:1], mneg[:], aff_a[:, 0:1], gb2_t[:],
                        op0=ALU.mult, op1=ALU.add)  # gb2 - mean*aff_a
                    nc.vector.memset(affb[:, 1:2], 0.0)
                    # fold affine scale into Wc slice for this head
                    nc.vector.tensor_scalar(wcs[j][:], wct[j][:],
                                            aff_a[:, 0:1], None, op0=ALU.mult)
                    gn_done[j] = affb
                    if j % 2 == 1:
                        emit_group(j // 2)
                return go

            def emit_group(g):
                if stage in ("h0", "heads"):
                    return
                js = (0, 1) if g == 0 else (2, 3)
                # rank-1 bias: bias[o] = sum_d wct[d,o] * affb[d], both heads
                bps = ps_z.tile([128, 16], F32, tag="z", name="bps")
                for ocb in range(8):
                    for i, j in enumerate(js):
                        nc.tensor.matmul(bps[:, ocb * 2:(ocb + 1) * 2],
                                         wct[j][:, ocb * 128:(ocb + 1) * 128],
                                         gn_done[j][:, 0:2],
                                         start=(i == 0), stop=(i == 1),
                                         skip_group_check=True)
                nc.vector.tensor_copy(bias_sb[g][:], bps[:])
                units = []
                for ocb in range(8):
                    for tb in range(T // 512):
                        units.append(make_po(g, js, ocb, tb))
                if g == 0:
                    P["fill"].extend(units)
                else:
                    for u in units:
                        u()

            def make_po(g, js, ocb, tb):
                def go():
                    po = ps_s.tile([128, 512], F32, tag="s", name="po")
                    for i, j in enumerate(js):
                        nc.tensor.matmul(
                            po[:],
                            wcs[j][:, ocb * 128:(ocb + 1) * 128],
                            ytr[j][:, tb * 512:(tb + 1) * 512],
                            start=(i == 0), stop=(i == 1),
                            skip_group_check=True)
                    ob = outsb[ocb][:, tb * 512:(tb + 1) * 512]
                    if g == 0 or (ocb + tb) % 2 == 0:
                        nc.vector.tensor_scalar(
                            ob, po[:],
                            bias_sb[g][:, 2 * ocb:2 * ocb + 1], None,
                            op0=ALU.add)
                    else:
                        nc.scalar.activation(
                            ob, po[:], AF.Identity,
                            bias=bias_sb[g][:, 2 * ocb:2 * ocb + 1])
                    if tb == 3:
                        ring = nc.sync if (g == 0 or ocb % 2 == 0)                             else nc.scalar
                        ring.dma_start(out_ds[g][ocb * 128:(ocb + 1) * 128, :],
                                       outsb[ocb][:])
                return go

            gn_done = {}

            LV = {"proj": 0, "sblk": 1, "zmm": 2, "comb": 3, "fin": 4,
                  "h0": 5, "heads": 6, "full": 7}[stage]
            nheads = 0 if LV == 0 else (1 if LV <= 5 else NH)
            for j in range(nheads):
                oc, po_ = divmod(j * HS, 128)
                s1p = wp.tile([128, 4], F32, tag="s1p", name="s1p", bufs=2)
                s2p = wp.tile([128, 4], F32, tag="s2p", name="s2p", bufs=2)
                head_post = make_gn(j, s1p, s2p)
                for qb in range(4):
                    zcb = wp.tile([128, 512], BF16, tag="zcb", name="zcb",
                                  bufs=2)
                    zp = [ps_z.tile([128, 260], F32, tag="z", name=f"zp{qq}")
                          for qq in range(4)]
                    for kt in range(4 * qb + 4):
                        diag = kt >= 4 * qb
                        qoff = (kt - 4 * qb) * 128 if diag else 0
                        ps = ps_s.tile([128, 2, 512], F32, tag="s", name="psS")
                        nc.tensor.matmul(
                            ps[:, 0, qoff:512],
                            k1t[oc][po_:po_ + HS, kt * 128:(kt + 1) * 128],
                            q1t[oc][po_:po_ + HS,
                                    qb * 512 + qoff:(qb + 1) * 512],
                            start=True, stop=True, skip_group_check=True)
                        nc.tensor.matmul(
                            ps[:, 1, qoff:512],
                            k2t[oc][po_:po_ + HS, kt * 128:(kt + 1) * 128],
                            q2t[oc][po_:po_ + HS,
                                    qb * 512 + qoff:(qb + 1) * 512],
                            start=True, stop=True, skip_group_check=True)
                        flush_pending()
                        e = wp.tile([128, 2, 512], BF16, tag="e", name="e",
                                    bufs=3)
                        nc.scalar.activation(e[:, :, qoff:512],
                                             ps[:, :, qoff:512],
                                             AF.Exp, scale=SCALE)
                        if diag:
                            # causal mask: zero e where q < k (multiplicative,
                            # safe: no inf in exp range here)
                            for m in range(2):
                                nc.vector.tensor_tensor(
                                    e[:, m, qoff:qoff + 128],
                                    e[:, m, qoff:qoff + 128],
                                    mask01_t[:], ALU.mult)

                        def make_z(j=j, qb=qb, kt=kt, e=e, zp=zp, zcb=zcb,
                                   s1p=s1p, s2p=s2p, head_post=head_post):
                            def go():
                                if LV < 2:
                                    return
                                for qt in range(max(kt, 4 * qb), 4 * qb + 4):
                                    qq = qt - 4 * qb
                                    for m in range(2):
                                        # start only the bank's first matmul:
                                        # start_tensor_calc marks the WHOLE
                                        # 2KB zero region pending-zero, so
                                        # z2's kt==0 write must rely on z1's
                                        # start (it zeroes on first touch).
                                        nc.tensor.matmul(
                                            zp[qq][:, m * 130:m * 130 + 130],
                                            e[:, m, qq * 128:qq * 128 + 128],
                                            vE[kt][:, j * VW:(j + 1) * VW],
                                            start=(kt == 0 and m == 0),
                                            stop=(kt == qt),
                                            skip_group_check=True)
                                if kt >= 4 * qb and LV >= 3:
                                    qq = kt - 4 * qb
                                    emit_combine(j, zp[qq], zcb, qq)
                                    if qq == 3 and LV >= 4:
                                        P["fin"].append(make_finish(
                                            j, qb, zcb, s1p, s2p,
                                            head_post if qb == 3 else None))
                            return go

                        P["z"] = make_z()

            # drain: last z block, last finish (incl. gn(3) -> emits group B)
            flush_pending()
            flush_pending()
            while P["fill"]:
                P["fill"].pop(0)()
            if stage != "full":
                # dump something defined to the outputs so the run completes
                for g, src_t in ((0, q1t[0]), (1, k1t[0])):
                    if LV >= 4:
                        src_t = ytr[0]
                    dmp = wp.tile([128, T], F32, tag="dmp", name="dmp", bufs=2)
                    nc.vector.tensor_copy(dmp[:], src_t[:])
                    nc.sync.dma_start(out_ds[g][0:128, :], dmp[:])

            wp_cm.__exit__(None, None, None)

    nc.compile()
    return nc


def _prep_inputs(inputs):
    bf = ml_dtypes.bfloat16
    x = np.asarray(inputs["x"], np.float32)
    Wq1 = np.asarray(inputs["Wq1"], np.float32)
    Wq2 = np.asarray(inputs["Wq2"], np.float32)
    Wk1 = np.asarray(inputs["Wk1"], np.float32)
    Wk2 = np.asarray(inputs["Wk2"], np.float32)
    Wv = np.asarray(inputs["Wv"], np.float32)
    Wc = np.asarray(inputs["Wc"], np.float32)
    gn_w = np.asarray(inputs["gn_w"], np.float32)
    gn_b = np.asarray(inputs["gn_b"], np.float32)
    gamma = np.asarray(inputs["gamma"], np.float32)

    def sig(v):
        return 1.0 / (1.0 + np.exp(-v))

    lam = (sig(np.asarray(inputs["lq1"], np.float32).reshape(H)
               * np.asarray(inputs["lk1"], np.float32).reshape(H))
           - sig(np.asarray(inputs["lq2"], np.float32).reshape(H)
                 * np.asarray(inputs["lk2"], np.float32).reshape(H))
           + LAMBDA_INIT)

    # S^T layout: mask rows k, cols q; keep where q >= k
    mask01 = (np.arange(128)[None, :] >= np.arange(128)[:, None]).astype(
        np.float32).astype(bf)
    ident = np.eye(128, dtype=np.float32).astype(bf)
    gg = (np.arange(128)[:, None] // 4 == np.arange(128)[None, :] // 4
          ).astype(np.float32)
    c1 = 1.0 - LAMBDA_INIT
    gw2 = (gn_w * gamma * c1).astype(np.float32).reshape(128, 1)
    gb2 = (gn_b * gamma * c1).astype(np.float32).reshape(128, 1)

    icon = np.zeros((128, 2), np.uint32)
    icon[:, 0] = 1
    icon[:, 1] = 0x5f375a00
    xTb = [np.ascontiguousarray(x[b].T).astype(bf) for b in range(B)]
    in_maps = []
    for core in range(N_CORES):
        b, hg = divmod(core, N_CORES // B)
        qs = hg * NH * HS          # 256-wide q/k slice
        vs = hg * NH * D           # 512-wide v / y2 slice
        lamn = np.repeat(-lam[hg * NH:(hg + 1) * NH].reshape(1, NH),
                         128, axis=0).astype(np.float32)
        in_maps.append({
            "xT": xTb[b],
            "wq1T": np.ascontiguousarray(Wq1[qs:qs + NH * HS, :].T).astype(bf),
            "wq2T": np.ascontiguousarray(Wq2[qs:qs + NH * HS, :].T).astype(bf),
            "wk1T": np.ascontiguousarray(Wk1[qs:qs + NH * HS, :].T).astype(bf),
            "wk2T": np.ascontiguousarray(Wk2[qs:qs + NH * HS, :].T).astype(bf),
            "wvT": np.ascontiguousarray(Wv[vs:vs + NH * D, :].T).astype(bf),
            "wcT": np.ascontiguousarray(Wc[:, vs:vs + NH * D].T).astype(bf),
            "mask01": mask01,
            "ident": ident,
            "gg": gg,
            "gw2": gw2,
            "gb2": gb2,
            "lamn": lamn,
            "icon": icon,
        })
    return in_maps


def kernel(**inputs):
    if "nc" not in _cache:
        _cache["nc"] = _build()
    nc = _cache["nc"]
    in_maps = _prep_inputs(inputs)
    res = bass_utils.run_bass_kernel_spmd(
        nc, in_maps, core_ids=list(range(N_CORES)),
        **_cache.get("run_kwargs", {}))
    _cache["last_result"] = res
    out = np.zeros((B, T, C), np.float32)
    for core in range(N_CORES):
        b = core // (N_CORES // B)
        out[b] += res.results[core]["outA"].astype(np.float32).T
        out[b] += res.results[core]["outB"].astype(np.float32).T
    return out
